# revision 20
# baseline (speedup 1.0000x reference)
"""Trainium2 Bass kernel for MoEConnectionProcessor (moe_routing).

Per row b (S=256, H=64, E=3):
  normed = LayerNorm(current_state)                      # stats on DVE, apply on ACT
  h      = gelu([normed, neighbor] @ w1 + b1)            # PE (transpose + matmul), erf on ACT
  w      = softmax(h @ w2 + b2)                          # PE + tanh-based exp (one ACT table set)
  out    = sum_e w[e] * expert_out_e                     # DVE + GPSIMD

Data parallel over 8 cores: rows sharded on host, weights replicated.
gamma/beta are folded into w1/b1 on the host; gelu's 0.5 is folded into w2.
exp(x) is computed as (1+tanh(x/2))/(1-tanh(x/2)) so Copy/Erf/Tanh all live in
the single `sigmoid_and_others` ACT table set (no per-tile table reloads).
1/sqrt(var+eps) is a linear seed + 3 Newton iterations on DVE (no Sqrt table).
"""

import sys

if "/opt/trn_rl_repo" not in sys.path:
    sys.path.insert(0, "/opt/trn_rl_repo")

import os
from functools import lru_cache

import numpy as np

N_CORES = 8
S = 256          # feature dim
H = 64           # gate hidden
E = 3            # experts
LN_EPS = 1e-5
P = 128          # SBUF partitions = rows per block
NBLK = 4         # row-blocks per supertile
ROWS_ST = P * NBLK
NCHUNK = 2 * S // P  # K chunks for mm1 (= 4)

# linear minimax-ish seed for 1/sqrt(v) on v in [0.3, 2.2]; 3 NR iters after.
RSQRT_A = -0.6015
RSQRT_B = 1.7516


@lru_cache(maxsize=16)
def _build(
    rows_per_core: int, add_b2: bool, add_b1: bool, n_rep: int = 1, abl: int = 4
):
    # abl: ablation level for benchmarking. 1=DMA only, 2=+combine,
    # 3=+LayerNorm, 4=full (gate MLP + softmax). Only 4 is correct.
    import concourse.bass as bass
    import concourse.mybir as mybir
    from concourse import bacc, tile
    from concourse.masks import make_identity

    f32 = mybir.dt.float32
    AF = mybir.ActivationFunctionType
    ALU = mybir.AluOpType

    assert rows_per_core % ROWS_ST == 0
    n_st = rows_per_core // ROWS_ST

    nc = bacc.Bacc(
        "TRN2", target_bir_lowering=False, debug=False, num_devices=N_CORES
    )
    cs = nc.dram_tensor("cs", [rows_per_core, S], f32, kind="ExternalInput")
    nb = nc.dram_tensor("nb", [rows_per_core, S], f32, kind="ExternalInput")
    e0 = nc.dram_tensor("e0", [rows_per_core, S], f32, kind="ExternalInput")
    e1 = nc.dram_tensor("e1", [rows_per_core, S], f32, kind="ExternalInput")
    e2 = nc.dram_tensor("e2", [rows_per_core, S], f32, kind="ExternalInput")
    w1 = nc.dram_tensor("w1", [P, NCHUNK * H], f32, kind="ExternalInput")
    b1x = nc.dram_tensor("b1x", [H, 1], f32, kind="ExternalInput")
    b1p = nc.dram_tensor("b1p", [H, 1], f32, kind="ExternalInput")
    w2 = nc.dram_tensor("w2", [H, E], f32, kind="ExternalInput")
    b2t = nc.dram_tensor("b2t", [NBLK * E], f32, kind="ExternalInput")
    comb = nc.dram_tensor("comb", [rows_per_core, S], f32, kind="ExternalOutput")
    # p-major so the single end-of-kernel store is fully contiguous;
    # kernel() transposes back to row order on the host.
    ew = nc.dram_tensor(
        "ew", [P, (rows_per_core // ROWS_ST) * NBLK * E], f32,
        kind="ExternalOutput",
    )

    # row r = t*ROWS_ST + p*NBLK + n: each partition reads/writes one
    # contiguous NBLK*S*4-byte run per DMA. The host keeps the natural
    # row order because loads and stores use the same mapping.
    pat = "(t p n) f -> t p n f"
    csr = cs.rearrange(pat, n=NBLK, p=P)
    nbr = nb.rearrange(pat, n=NBLK, p=P)
    e0r = e0.rearrange(pat, n=NBLK, p=P)
    e1r = e1.rearrange(pat, n=NBLK, p=P)
    e2r = e2.rearrange(pat, n=NBLK, p=P)
    combr = comb.rearrange(pat, n=NBLK, p=P)


    with tile.TileContext(nc) as tc:
        from contextlib import ExitStack

        with ExitStack() as ctx:
            const = ctx.enter_context(tc.tile_pool(name="const", bufs=1))
            pin = ctx.enter_context(tc.tile_pool(name="pin", bufs=3))
            pnorm = ctx.enter_context(tc.tile_pool(name="pnorm", bufs=2))
            pxt = ctx.enter_context(tc.tile_pool(name="pxt", bufs=2))
            phg = ctx.enter_context(tc.tile_pool(name="phg", bufs=2))
            pout = ctx.enter_context(tc.tile_pool(name="pout", bufs=2))
            psmall = ctx.enter_context(tc.tile_pool(name="psmall", bufs=2))
            pcomb = ctx.enter_context(tc.tile_pool(name="pcomb", bufs=2))
            ppt = ctx.enter_context(tc.tile_pool(name="ppt", bufs=2, space="PSUM"))
            pph = ctx.enter_context(tc.tile_pool(name="pph", bufs=2, space="PSUM"))
            ppl = ctx.enter_context(tc.tile_pool(name="ppl", bufs=2, space="PSUM"))

            identity = const.tile([P, P], f32)
            make_identity(nc, identity)
            w1sb = const.tile([P, NCHUNK * H], f32)
            nc.sync.dma_start(out=w1sb, in_=w1[:, :])
            w2sb = const.tile([H, E], f32)
            nc.sync.dma_start(out=w2sb, in_=w2[:, :])
            b1xsb = const.tile([H, 1], f32)
            nc.sync.dma_start(out=b1xsb, in_=b1x[:, :])
            if add_b1:
                b1psb = const.tile([H, 1], f32)
                nc.sync.dma_start(out=b1psb, in_=b1p[:, :])
            if add_b2:
                b2sb = const.tile([P, NBLK, E], f32)
                b2_bcast = bass.AP(
                    tensor=b2t, offset=0, ap=[[0, P], [1, NBLK * E]]
                )
                nc.gpsimd.dma_start(out=b2sb, in_=b2_bcast)

            wnc = None
            if abl < 4:
                wnc = const.tile([P, NBLK, E], f32)
                nc.vector.memset(wnc, 0.33)
            wn_all = const.tile([P, n_st, NBLK, E], f32)
            if abl < 4:
                nc.vector.memset(wn_all, 0.0)

            from contextlib import nullcontext

            rep_ctx = tc.For_i(0, n_rep, 1) if n_rep > 1 else nullcontext()
            with rep_ctx:
                for t in range(n_st):
                    _supertile(
                        nc, tc, mybir, bass, t,
                        pin, pnorm, pxt, phg, pout, psmall, pcomb,
                        ppt, pph, ppl,
                        identity, w1sb, w2sb, b1xsb,
                        b1psb if add_b1 else None,
                        b2sb if add_b2 else None,
                        csr, nbr, e0r, e1r, e2r, combr, wn_all,
                        abl, wnc,
                    )
                nc.sync.dma_start(out=ew[:, :], in_=wn_all)

    nc.compile()
    return nc


def _supertile(
    nc, tc, mybir, bass, t,
    pin, pnorm, pxt, phg, pout, psmall, pcomb,
    ppt, pph, ppl,
    identity, w1sb, w2sb, b1xsb, b1psb, b2sb,
    csr, nbr, e0r, e1r, e2r, combr, wn_all,
    abl=4, wnc=None,
):
    f32 = mybir.dt.float32
    AF = mybir.ActivationFunctionType
    ALU = mybir.AluOpType
    add_b1 = b1psb is not None
    add_b2 = b2sb is not None
    if True:
            if True:
                xs = pin.tile([P, NBLK, S], f32, tag="xs")
                nc.sync.dma_start(out=xs, in_=csr[t])
                nbt = pin.tile([P, NBLK, S], f32, tag="nbt")
                nc.sync.dma_start(out=nbt, in_=nbr[t])
                et0 = pin.tile([P, NBLK, S], f32, tag="et0")
                nc.sync.dma_start(out=et0, in_=e0r[t])
                et1 = pin.tile([P, NBLK, S], f32, tag="et1")
                nc.sync.dma_start(out=et1, in_=e1r[t])
                et2 = pin.tile([P, NBLK, S], f32, tag="et2")
                nc.sync.dma_start(out=et2, in_=e2r[t])

                if abl <= 1:
                    # DMA only: store a load-dependent tile straight back
                    nc.sync.dma_start(out=combr[t], in_=et0)
                    return
                if abl == 2:
                    _combine(nc, mybir, t, pin, pout, pcomb, wnc,
                             et0, et1, et2, combr)
                    return

                # --- LayerNorm stats ---
                stats = psmall.tile([P, NBLK, 6], f32, tag="stats")
                # walrus requires exactly 6 output elems/partition per BNStats
                for n in range(NBLK):
                    nc.vector.bn_stats(out=stats[:, n], in_=xs[:, n])
                mv = psmall.tile([P, NBLK, 2], f32, tag="mv")
                for n in range(NBLK):
                    nc.vector.bn_aggr(out=mv[:, n], in_=stats[:, n])
                ve = psmall.tile([P, NBLK], f32, tag="ve")
                nc.vector.tensor_scalar_add(out=ve, in0=mv[:, :, 1], scalar1=LN_EPS)
                y = psmall.tile([P, NBLK], f32, tag="y")
                nc.vector.tensor_scalar(
                    out=y, in0=ve, scalar1=RSQRT_A, scalar2=RSQRT_B,
                    op0=ALU.mult, op1=ALU.add,
                )
                tnr = psmall.tile([P, NBLK], f32, tag="tnr")
                for _ in range(3):
                    nc.vector.tensor_mul(out=tnr, in0=y, in1=y)
                    nc.vector.tensor_mul(out=tnr, in0=tnr, in1=ve)
                    nc.vector.tensor_scalar(
                        out=tnr, in0=tnr, scalar1=-0.5, scalar2=1.5,
                        op0=ALU.mult, op1=ALU.add,
                    )
                    nc.vector.tensor_mul(out=y, in0=y, in1=tnr)
                nm = psmall.tile([P, NBLK], f32, tag="nm")
                nc.vector.tensor_scalar_mul(out=nm, in0=mv[:, :, 0], scalar1=-1.0)
                nc.vector.tensor_mul(out=nm, in0=nm, in1=y)

                # --- LN apply: normed = x*rstd + (-mu*rstd) on ACT ---
                normed = pnorm.tile([P, NBLK, S], f32, tag="normed")
                for n in range(NBLK):
                    nc.scalar.activation(
                        out=normed[:, n], in_=xs[:, n], func=AF.Identity,
                        bias=nm[:, n : n + 1], scale=y[:, n : n + 1],
                    )

                if abl == 3:
                    _combine(nc, mybir, t, pin, pout, pcomb, wnc,
                             et0, et1, et2, combr, extra_dep=normed)
                    return

                # --- transpose [normed | neighbor] to feature-major ---
                xt = pxt.tile([P, NBLK, NCHUNK, P], f32, tag="xt")
                for n in range(NBLK):
                    pt = ppt.tile([P, NCHUNK, P], f32, tag="pt")
                    for k in range(NCHUNK):
                        if k < NCHUNK // 2:
                            src = normed[:, n, k * P : (k + 1) * P]
                        else:
                            kk = k - NCHUNK // 2
                            src = nbt[:, n, kk * P : (kk + 1) * P]
                        nc.tensor.transpose(pt[:, k], src, identity)
                    nc.scalar.activation(out=xt[:, n], in_=pt, func=AF.Copy)

                # --- mm1: h_pre^T [H, NBLK, P] ---
                ph = pph.tile([H, NBLK, P], f32, tag="ph")
                for k in range(NCHUNK):
                    nc.tensor.matmul(
                        ph, w1sb[:, k * H : (k + 1) * H], xt[:, :, k, :],
                        start=(k == 0), stop=(k == NCHUNK - 1),
                    )

                # --- gelu(x) = x*(1+erf(x/sqrt(2)))/2 ; the 0.5 is folded into w2 ---
                esb = phg.tile([H, NBLK, P], f32, tag="esb")
                nc.scalar.activation(
                    out=esb, in_=ph, func=AF.Erf, bias=b1xsb, scale=0.70710678
                )
                if add_b1:
                    xg = phg.tile([H, NBLK, P], f32, tag="xg")
                    nc.scalar.activation(out=xg, in_=ph, func=AF.Identity, bias=b1psb)
                    xsrc = xg
                else:
                    xsrc = ph
                tg = phg.tile([H, NBLK, P], f32, tag="tg")
                nc.vector.tensor_mul(out=tg, in0=xsrc, in1=esb)
                hg = phg.tile([H, NBLK, P], f32, tag="hg")
                nc.vector.tensor_add(out=hg, in0=tg, in1=xsrc)

                # --- mm2: logits [P, NBLK, E] ---
                pl = ppl.tile([P, NBLK, E], f32, tag="pl")
                for n in range(NBLK):
                    nc.tensor.matmul(
                        pl[:, n], hg[:, n, :], w2sb, start=True, stop=True
                    )

                # --- softmax via tanh-exp ---
                lcl = psmall.tile([P, NBLK, E], f32, tag="lcl")
                if add_b2:
                    nc.vector.tensor_add(out=lcl, in0=pl, in1=b2sb)
                    nc.vector.tensor_scalar(
                        out=lcl, in0=lcl, scalar1=16.0, scalar2=None, op0=ALU.min
                    )
                else:
                    nc.vector.tensor_scalar(
                        out=lcl, in0=pl, scalar1=16.0, scalar2=None, op0=ALU.min
                    )
                th = psmall.tile([P, NBLK, E], f32, tag="th")
                nc.scalar.activation(out=th, in_=lcl, func=AF.Tanh, scale=0.5)
                num = psmall.tile([P, NBLK, E], f32, tag="num")
                nc.vector.tensor_scalar_add(out=num, in0=th, scalar1=1.0)
                den = psmall.tile([P, NBLK, E], f32, tag="den")
                nc.vector.tensor_scalar(
                    out=den, in0=th, scalar1=-1.0, scalar2=1.0,
                    op0=ALU.mult, op1=ALU.add,
                )
                rden = psmall.tile([P, NBLK, E], f32, tag="rden")
                nc.vector.reciprocal(out=rden, in_=den)
                ex = psmall.tile([P, NBLK, E], f32, tag="ex")
                nc.vector.tensor_mul(out=ex, in0=num, in1=rden)
                ssum = psmall.tile([P, NBLK], f32, tag="ssum")
                nc.vector.tensor_reduce(
                    out=ssum, in_=ex, axis=mybir.AxisListType.X, op=ALU.add
                )
                rs = psmall.tile([P, NBLK], f32, tag="rs")
                nc.vector.reciprocal(out=rs, in_=ssum)
                wn = wn_all[:, t]
                for n in range(NBLK):
                    nc.vector.tensor_scalar_mul(
                        out=wn[:, n], in0=ex[:, n], scalar1=rs[:, n : n + 1]
                    )
                _combine(nc, mybir, t, pin, pout, pcomb, wn,
                         et0, et1, et2, combr)


def _combine(nc, mybir, t, pin, pout, pcomb, wn, et0, et1, et2,
             combr, extra_dep=None):
    f32 = mybir.dt.float32
    outc = pout.tile([P, NBLK, S], f32, tag="outc")
    m1 = pcomb.tile([P, NBLK, S], f32, tag="m1")
    m2 = pcomb.tile([P, NBLK, S], f32, tag="m2")
    for n in range(NBLK):
        nc.gpsimd.tensor_scalar_mul(
            out=m1[:, n], in0=et1[:, n], scalar1=wn[:, n, 1:2]
        )
        nc.gpsimd.tensor_scalar_mul(
            out=m2[:, n], in0=et2[:, n], scalar1=wn[:, n, 2:3]
        )
        src0 = et0[:, n] if extra_dep is None else extra_dep[:, n]
        nc.vector.tensor_scalar_mul(
            out=outc[:, n], in0=src0, scalar1=wn[:, n, 0:1]
        )
        nc.vector.tensor_add(out=outc[:, n], in0=outc[:, n], in1=m1[:, n])
        nc.vector.tensor_add(out=outc[:, n], in0=outc[:, n], in1=m2[:, n])
    nc.sync.dma_start(out=combr[t], in_=outc)


def kernel(
    current_state,
    neighbor_activity,
    expert_out_0,
    expert_out_1,
    expert_out_2,
    ln_gamma,
    ln_beta,
    w1,
    b1,
    w2,
    b2,
):
    from concourse.bass_utils import run_bass_kernel_spmd

    f32 = np.float32
    cs = np.ascontiguousarray(np.asarray(current_state, dtype=f32))
    nb = np.ascontiguousarray(np.asarray(neighbor_activity, dtype=f32))
    e0 = np.ascontiguousarray(np.asarray(expert_out_0, dtype=f32))
    e1 = np.ascontiguousarray(np.asarray(expert_out_1, dtype=f32))
    e2 = np.ascontiguousarray(np.asarray(expert_out_2, dtype=f32))
    gamma = np.asarray(ln_gamma, dtype=f32)
    beta = np.asarray(ln_beta, dtype=f32)
    w1_ = np.asarray(w1, dtype=f32)
    b1_ = np.asarray(b1, dtype=f32)
    w2_ = np.asarray(w2, dtype=f32)
    b2_ = np.asarray(b2, dtype=f32)

    B = cs.shape[0]
    assert B % (N_CORES * ROWS_ST) == 0, B
    rows = B // N_CORES

    # host-side folds (exact; gamma/beta fold into w1/b1, gelu's 0.5 into w2)
    s_dim = cs.shape[1]
    w1_eff = np.concatenate(
        [gamma[:, None] * w1_[:s_dim], w1_[s_dim:]], axis=0
    ).astype(f32)
    b1_eff = (b1_ + beta @ w1_[:s_dim]).astype(f32)
    w2_eff = (0.5 * w2_).astype(f32)
    # chunk-major layout for direct [P, NCHUNK*H] SBUF load
    w1_dev = np.ascontiguousarray(
        w1_eff.reshape(NCHUNK, P, H).transpose(1, 0, 2).reshape(P, NCHUNK * H)
    )
    b1x_dev = np.ascontiguousarray((b1_eff * np.float32(0.70710678))[:, None])
    b1p_dev = np.ascontiguousarray(b1_eff[:, None])
    b2_dev = np.ascontiguousarray(np.tile(b2_, NBLK))

    add_b1 = bool(np.any(b1_eff != 0))
    add_b2 = bool(np.any(b2_ != 0))
    nc = _build(rows, add_b2, add_b1)

    in_maps = []
    for c in range(N_CORES):
        sl = slice(c * rows, (c + 1) * rows)
        in_maps.append(
            {
                "cs": cs[sl],
                "nb": nb[sl],
                "e0": e0[sl],
                "e1": e1[sl],
                "e2": e2[sl],
                "w1": w1_dev,
                "b1x": b1x_dev,
                "b1p": b1p_dev,
                "w2": w2_eff,
                "b2t": b2_dev,
            }
        )

    trace = bool(int(os.environ.get("BASS_KERNEL_TRACE", "0")))
    res = run_bass_kernel_spmd(nc, in_maps, list(range(N_CORES)), trace=trace)
    if trace:
        kernel.last_results = res

    combined = np.concatenate([res.results[c]["comb"] for c in range(N_CORES)], 0)
    n_st = rows // ROWS_ST
    weights = np.concatenate(
        [
            res.results[c]["ew"]
            .reshape(P, n_st, NBLK, E)
            .transpose(1, 0, 2, 3)
            .reshape(rows, E)
            for c in range(N_CORES)
        ],
        0,
    )
    return combined, weights


# revision 21
# speedup vs baseline: 3.2100x; 3.2100x over previous
"""Trainium2 Bass kernel for MoEConnectionProcessor (moe_routing).

Per row b (S=256, H=64, E=3):
  normed = LayerNorm(current_state)                      # stats on DVE, apply on ACT
  h      = gelu([normed, neighbor] @ w1 + b1)            # PE (transpose + matmul), erf on ACT
  w      = softmax(h @ w2 + b2)                          # PE + tanh-based exp (one ACT table set)
  out    = sum_e w[e] * expert_out_e                     # DVE + GPSIMD

Data parallel over 8 cores: rows sharded on host, weights replicated.
gamma/beta are folded into w1/b1 on the host; gelu's 0.5 is folded into w2.
exp(x) is computed as (1+tanh(x/2))/(1-tanh(x/2)) so Copy/Erf/Tanh all live in
the single `sigmoid_and_others` ACT table set (no per-tile table reloads).
1/sqrt(var+eps) is a linear seed + 3 Newton iterations on DVE (no Sqrt table).
"""

import sys

if "/opt/trn_rl_repo" not in sys.path:
    sys.path.insert(0, "/opt/trn_rl_repo")

import os
from functools import lru_cache

import numpy as np

N_CORES = 8
S = 256          # feature dim
H = 64           # gate hidden
E = 3            # experts
LN_EPS = 1e-5
P = 128          # SBUF partitions = rows per block
NBLK = 4         # row-blocks per supertile
ROWS_ST = P * NBLK
NCHUNK = 2 * S // P  # K chunks for mm1 (= 4)

# linear minimax-ish seed for 1/sqrt(v) on v in [0.3, 2.2]; 3 NR iters after.
RSQRT_A = -0.6015
RSQRT_B = 1.7516


@lru_cache(maxsize=16)
def _build(
    rows_per_core: int, add_b2: bool, add_b1: bool, n_rep: int = 1, abl: int = 4,
    cmode: int = 2,
):
    # abl: ablation level for benchmarking. 1=DMA only, 2=+combine,
    # 3=+LayerNorm, 4=full (gate MLP + softmax). Only 4 is correct.
    import concourse.bass as bass
    import concourse.mybir as mybir
    from concourse import bacc, tile
    from concourse.masks import make_identity

    f32 = mybir.dt.float32
    AF = mybir.ActivationFunctionType
    ALU = mybir.AluOpType

    assert rows_per_core % ROWS_ST == 0
    n_st = rows_per_core // ROWS_ST

    nc = bacc.Bacc(
        "TRN2", target_bir_lowering=False, debug=False, num_devices=N_CORES
    )
    cs = nc.dram_tensor("cs", [rows_per_core, S], f32, kind="ExternalInput")
    nb = nc.dram_tensor("nb", [rows_per_core, S], f32, kind="ExternalInput")
    e0 = nc.dram_tensor("e0", [rows_per_core, S], f32, kind="ExternalInput")
    e1 = nc.dram_tensor("e1", [rows_per_core, S], f32, kind="ExternalInput")
    e2 = nc.dram_tensor("e2", [rows_per_core, S], f32, kind="ExternalInput")
    w1 = nc.dram_tensor("w1", [P, NCHUNK * H], f32, kind="ExternalInput")
    b1x = nc.dram_tensor("b1x", [H, 1], f32, kind="ExternalInput")
    b1p = nc.dram_tensor("b1p", [H, 1], f32, kind="ExternalInput")
    w2 = nc.dram_tensor("w2", [H, E], f32, kind="ExternalInput")
    b2t = nc.dram_tensor("b2t", [NBLK * E], f32, kind="ExternalInput")
    comb = nc.dram_tensor("comb", [rows_per_core, S], f32, kind="ExternalOutput")
    # p-major so the single end-of-kernel store is fully contiguous;
    # kernel() transposes back to row order on the host.
    ew = nc.dram_tensor(
        "ew", [P, (rows_per_core // ROWS_ST) * NBLK * E], f32,
        kind="ExternalOutput",
    )

    # row r = t*ROWS_ST + p*NBLK + n: each partition reads/writes one
    # contiguous NBLK*S*4-byte run per DMA. The host keeps the natural
    # row order because loads and stores use the same mapping.
    pat = "(t p n) f -> t p n f"
    csr = cs.rearrange(pat, n=NBLK, p=P)
    nbr = nb.rearrange(pat, n=NBLK, p=P)
    e0r = e0.rearrange(pat, n=NBLK, p=P)
    e1r = e1.rearrange(pat, n=NBLK, p=P)
    e2r = e2.rearrange(pat, n=NBLK, p=P)
    combr = comb.rearrange(pat, n=NBLK, p=P)


    with tile.TileContext(nc) as tc:
        from contextlib import ExitStack

        with ExitStack() as ctx:
            const = ctx.enter_context(tc.tile_pool(name="const", bufs=1))
            pin = ctx.enter_context(tc.tile_pool(name="pin", bufs=3))
            pnorm = ctx.enter_context(tc.tile_pool(name="pnorm", bufs=2))
            pxt = ctx.enter_context(tc.tile_pool(name="pxt", bufs=2))
            phg = ctx.enter_context(tc.tile_pool(name="phg", bufs=2))
            pout = ctx.enter_context(tc.tile_pool(name="pout", bufs=2))
            psmall = ctx.enter_context(tc.tile_pool(name="psmall", bufs=2))
            pcomb = ctx.enter_context(tc.tile_pool(name="pcomb", bufs=2))
            ppt = ctx.enter_context(tc.tile_pool(name="ppt", bufs=2, space="PSUM"))
            pph = ctx.enter_context(tc.tile_pool(name="pph", bufs=2, space="PSUM"))
            ppl = ctx.enter_context(tc.tile_pool(name="ppl", bufs=2, space="PSUM"))

            identity = const.tile([P, P], f32)
            make_identity(nc, identity)
            w1sb = const.tile([P, NCHUNK * H], f32)
            nc.sync.dma_start(out=w1sb, in_=w1[:, :])
            w2sb = const.tile([H, E], f32)
            nc.sync.dma_start(out=w2sb, in_=w2[:, :])
            b1xsb = const.tile([H, 1], f32)
            nc.sync.dma_start(out=b1xsb, in_=b1x[:, :])
            if add_b1:
                b1psb = const.tile([H, 1], f32)
                nc.sync.dma_start(out=b1psb, in_=b1p[:, :])
            if add_b2:
                b2sb = const.tile([P, NBLK, E], f32)
                b2_bcast = bass.AP(
                    tensor=b2t, offset=0, ap=[[0, P], [1, NBLK * E]]
                )
                nc.gpsimd.dma_start(out=b2sb, in_=b2_bcast)

            wnc = None
            if abl < 4:
                wnc = const.tile([P, NBLK, E], f32)
                nc.vector.memset(wnc, 0.33)
            wn_all = const.tile([P, n_st, NBLK, E], f32)
            if abl < 4:
                nc.vector.memset(wn_all, 0.0)

            from contextlib import nullcontext

            rep_ctx = tc.For_i(0, n_rep, 1) if n_rep > 1 else nullcontext()
            with rep_ctx:
                for t in range(n_st):
                    _supertile(
                        nc, tc, mybir, bass, t,
                        pin, pnorm, pxt, phg, pout, psmall, pcomb,
                        ppt, pph, ppl,
                        identity, w1sb, w2sb, b1xsb,
                        b1psb if add_b1 else None,
                        b2sb if add_b2 else None,
                        csr, nbr, e0r, e1r, e2r, combr, wn_all,
                        abl, wnc, cmode,
                    )
                nc.sync.dma_start(out=ew[:, :], in_=wn_all)

    nc.compile()
    return nc


def _supertile(
    nc, tc, mybir, bass, t,
    pin, pnorm, pxt, phg, pout, psmall, pcomb,
    ppt, pph, ppl,
    identity, w1sb, w2sb, b1xsb, b1psb, b2sb,
    csr, nbr, e0r, e1r, e2r, combr, wn_all,
    abl=4, wnc=None, cmode=2,
):
    f32 = mybir.dt.float32
    AF = mybir.ActivationFunctionType
    ALU = mybir.AluOpType
    add_b1 = b1psb is not None
    add_b2 = b2sb is not None
    if True:
            if True:
                xs = pin.tile([P, NBLK, S], f32, tag="xs")
                nc.sync.dma_start(out=xs, in_=csr[t])
                nbt = pin.tile([P, NBLK, S], f32, tag="nbt")
                nc.sync.dma_start(out=nbt, in_=nbr[t])
                et0 = pin.tile([P, NBLK, S], f32, tag="et0")
                nc.sync.dma_start(out=et0, in_=e0r[t])
                et1 = pin.tile([P, NBLK, S], f32, tag="et1")
                nc.sync.dma_start(out=et1, in_=e1r[t])
                et2 = pin.tile([P, NBLK, S], f32, tag="et2")
                nc.sync.dma_start(out=et2, in_=e2r[t])

                if abl <= 1:
                    # DMA only: store a load-dependent tile straight back
                    nc.sync.dma_start(out=combr[t], in_=et0)
                    return
                if abl == 2:
                    _combine(nc, mybir, t, pin, pout, pcomb, wnc,
                             et0, et1, et2, combr, cmode=cmode)
                    return

                # --- LayerNorm stats ---
                stats = psmall.tile([P, NBLK, 6], f32, tag="stats")
                # walrus requires exactly 6 output elems/partition per BNStats
                for n in range(NBLK):
                    nc.vector.bn_stats(out=stats[:, n], in_=xs[:, n])
                mv = psmall.tile([P, NBLK, 2], f32, tag="mv")
                for n in range(NBLK):
                    nc.vector.bn_aggr(out=mv[:, n], in_=stats[:, n])
                ve = psmall.tile([P, NBLK], f32, tag="ve")
                nc.vector.tensor_scalar_add(out=ve, in0=mv[:, :, 1], scalar1=LN_EPS)
                y = psmall.tile([P, NBLK], f32, tag="y")
                nc.vector.tensor_scalar(
                    out=y, in0=ve, scalar1=RSQRT_A, scalar2=RSQRT_B,
                    op0=ALU.mult, op1=ALU.add,
                )
                tnr = psmall.tile([P, NBLK], f32, tag="tnr")
                for _ in range(3):
                    nc.vector.tensor_mul(out=tnr, in0=y, in1=y)
                    nc.vector.tensor_mul(out=tnr, in0=tnr, in1=ve)
                    nc.vector.tensor_scalar(
                        out=tnr, in0=tnr, scalar1=-0.5, scalar2=1.5,
                        op0=ALU.mult, op1=ALU.add,
                    )
                    nc.vector.tensor_mul(out=y, in0=y, in1=tnr)
                nm = psmall.tile([P, NBLK], f32, tag="nm")
                nc.vector.tensor_scalar_mul(out=nm, in0=mv[:, :, 0], scalar1=-1.0)
                nc.vector.tensor_mul(out=nm, in0=nm, in1=y)

                # --- LN apply: normed = x*rstd + (-mu*rstd) on ACT ---
                normed = pnorm.tile([P, NBLK, S], f32, tag="normed")
                for n in range(NBLK):
                    nc.scalar.activation(
                        out=normed[:, n], in_=xs[:, n], func=AF.Identity,
                        bias=nm[:, n : n + 1], scale=y[:, n : n + 1],
                    )

                if abl == 3:
                    _combine(nc, mybir, t, pin, pout, pcomb, wnc,
                             et0, et1, et2, combr, extra_dep=normed,
                             cmode=cmode)
                    return

                # --- transpose [normed | neighbor] to feature-major ---
                xt = pxt.tile([P, NBLK, NCHUNK, P], f32, tag="xt")
                for n in range(NBLK):
                    pt = ppt.tile([P, NCHUNK, P], f32, tag="pt")
                    for k in range(NCHUNK):
                        if k < NCHUNK // 2:
                            src = normed[:, n, k * P : (k + 1) * P]
                        else:
                            kk = k - NCHUNK // 2
                            src = nbt[:, n, kk * P : (kk + 1) * P]
                        nc.tensor.transpose(pt[:, k], src, identity)
                    nc.scalar.activation(out=xt[:, n], in_=pt, func=AF.Copy)

                # --- mm1: h_pre^T [H, NBLK, P] ---
                ph = pph.tile([H, NBLK, P], f32, tag="ph")
                for k in range(NCHUNK):
                    nc.tensor.matmul(
                        ph, w1sb[:, k * H : (k + 1) * H], xt[:, :, k, :],
                        start=(k == 0), stop=(k == NCHUNK - 1),
                    )

                # --- gelu(x) = x*(1+erf(x/sqrt(2)))/2 ; the 0.5 is folded into w2 ---
                esb = phg.tile([H, NBLK, P], f32, tag="esb")
                nc.scalar.activation(
                    out=esb, in_=ph, func=AF.Erf, bias=b1xsb, scale=0.70710678
                )
                if add_b1:
                    xg = phg.tile([H, NBLK, P], f32, tag="xg")
                    nc.scalar.activation(out=xg, in_=ph, func=AF.Identity, bias=b1psb)
                    xsrc = xg
                else:
                    xsrc = ph
                tg = phg.tile([H, NBLK, P], f32, tag="tg")
                nc.vector.tensor_mul(out=tg, in0=xsrc, in1=esb)
                hg = phg.tile([H, NBLK, P], f32, tag="hg")
                nc.vector.tensor_add(out=hg, in0=tg, in1=xsrc)

                # --- mm2: logits [P, NBLK, E] ---
                pl = ppl.tile([P, NBLK, E], f32, tag="pl")
                for n in range(NBLK):
                    nc.tensor.matmul(
                        pl[:, n], hg[:, n, :], w2sb, start=True, stop=True
                    )

                # --- softmax via tanh-exp ---
                lcl = psmall.tile([P, NBLK, E], f32, tag="lcl")
                if add_b2:
                    nc.vector.tensor_add(out=lcl, in0=pl, in1=b2sb)
                    nc.vector.tensor_scalar(
                        out=lcl, in0=lcl, scalar1=16.0, scalar2=None, op0=ALU.min
                    )
                else:
                    nc.vector.tensor_scalar(
                        out=lcl, in0=pl, scalar1=16.0, scalar2=None, op0=ALU.min
                    )
                th = psmall.tile([P, NBLK, E], f32, tag="th")
                nc.scalar.activation(out=th, in_=lcl, func=AF.Tanh, scale=0.5)
                num = psmall.tile([P, NBLK, E], f32, tag="num")
                nc.vector.tensor_scalar_add(out=num, in0=th, scalar1=1.0)
                den = psmall.tile([P, NBLK, E], f32, tag="den")
                nc.vector.tensor_scalar(
                    out=den, in0=th, scalar1=-1.0, scalar2=1.0,
                    op0=ALU.mult, op1=ALU.add,
                )
                rden = psmall.tile([P, NBLK, E], f32, tag="rden")
                nc.vector.reciprocal(out=rden, in_=den)
                ex = psmall.tile([P, NBLK, E], f32, tag="ex")
                nc.vector.tensor_mul(out=ex, in0=num, in1=rden)
                ssum = psmall.tile([P, NBLK], f32, tag="ssum")
                nc.vector.tensor_reduce(
                    out=ssum, in_=ex, axis=mybir.AxisListType.X, op=ALU.add
                )
                rs = psmall.tile([P, NBLK], f32, tag="rs")
                nc.vector.reciprocal(out=rs, in_=ssum)
                wn = wn_all[:, t]
                for n in range(NBLK):
                    nc.vector.tensor_scalar_mul(
                        out=wn[:, n], in0=ex[:, n], scalar1=rs[:, n : n + 1]
                    )
                _combine(nc, mybir, t, pin, pout, pcomb, wn,
                         et0, et1, et2, combr, cmode=cmode)


def _combine(nc, mybir, t, pin, pout, pcomb, wn, et0, et1, et2,
             combr, extra_dep=None, cmode=2):
    f32 = mybir.dt.float32
    AF = mybir.ActivationFunctionType
    outc = pout.tile([P, NBLK, S], f32, tag="outc")
    m1 = pcomb.tile([P, NBLK, S], f32, tag="m1")
    m2 = pcomb.tile([P, NBLK, S], f32, tag="m2")
    for n in range(NBLK):
        # m1 = w1*E1, m2 = w2*E2 on a helper engine; m0 + adds on DVE
        if cmode == 0:
            nc.gpsimd.tensor_scalar_mul(
                out=m1[:, n], in0=et1[:, n], scalar1=wn[:, n, 1:2]
            )
            nc.gpsimd.tensor_scalar_mul(
                out=m2[:, n], in0=et2[:, n], scalar1=wn[:, n, 2:3]
            )
        elif cmode == 1:
            nc.vector.tensor_scalar_mul(
                out=m1[:, n], in0=et1[:, n], scalar1=wn[:, n, 1:2]
            )
            nc.vector.tensor_scalar_mul(
                out=m2[:, n], in0=et2[:, n], scalar1=wn[:, n, 2:3]
            )
        elif cmode == 2:
            nc.scalar.activation(
                out=m1[:, n], in_=et1[:, n], func=AF.Identity,
                scale=wn[:, n, 1:2],
            )
            nc.scalar.activation(
                out=m2[:, n], in_=et2[:, n], func=AF.Identity,
                scale=wn[:, n, 2:3],
            )
        elif cmode == 3:
            nc.scalar.activation(
                out=m1[:, n], in_=et1[:, n], func=AF.Identity,
                scale=wn[:, n, 1:2],
            )
            nc.gpsimd.tensor_scalar_mul(
                out=m2[:, n], in0=et2[:, n], scalar1=wn[:, n, 2:3]
            )
        src0 = et0[:, n] if extra_dep is None else extra_dep[:, n]
        nc.vector.tensor_scalar_mul(
            out=outc[:, n], in0=src0, scalar1=wn[:, n, 0:1]
        )
        nc.vector.tensor_add(out=outc[:, n], in0=outc[:, n], in1=m1[:, n])
        nc.vector.tensor_add(out=outc[:, n], in0=outc[:, n], in1=m2[:, n])
    nc.sync.dma_start(out=combr[t], in_=outc)


def kernel(
    current_state,
    neighbor_activity,
    expert_out_0,
    expert_out_1,
    expert_out_2,
    ln_gamma,
    ln_beta,
    w1,
    b1,
    w2,
    b2,
):
    from concourse.bass_utils import run_bass_kernel_spmd

    f32 = np.float32
    cs = np.ascontiguousarray(np.asarray(current_state, dtype=f32))
    nb = np.ascontiguousarray(np.asarray(neighbor_activity, dtype=f32))
    e0 = np.ascontiguousarray(np.asarray(expert_out_0, dtype=f32))
    e1 = np.ascontiguousarray(np.asarray(expert_out_1, dtype=f32))
    e2 = np.ascontiguousarray(np.asarray(expert_out_2, dtype=f32))
    gamma = np.asarray(ln_gamma, dtype=f32)
    beta = np.asarray(ln_beta, dtype=f32)
    w1_ = np.asarray(w1, dtype=f32)
    b1_ = np.asarray(b1, dtype=f32)
    w2_ = np.asarray(w2, dtype=f32)
    b2_ = np.asarray(b2, dtype=f32)

    B = cs.shape[0]
    assert B % (N_CORES * ROWS_ST) == 0, B
    rows = B // N_CORES

    # host-side folds (exact; gamma/beta fold into w1/b1, gelu's 0.5 into w2)
    s_dim = cs.shape[1]
    w1_eff = np.concatenate(
        [gamma[:, None] * w1_[:s_dim], w1_[s_dim:]], axis=0
    ).astype(f32)
    b1_eff = (b1_ + beta @ w1_[:s_dim]).astype(f32)
    w2_eff = (0.5 * w2_).astype(f32)
    # chunk-major layout for direct [P, NCHUNK*H] SBUF load
    w1_dev = np.ascontiguousarray(
        w1_eff.reshape(NCHUNK, P, H).transpose(1, 0, 2).reshape(P, NCHUNK * H)
    )
    b1x_dev = np.ascontiguousarray((b1_eff * np.float32(0.70710678))[:, None])
    b1p_dev = np.ascontiguousarray(b1_eff[:, None])
    b2_dev = np.ascontiguousarray(np.tile(b2_, NBLK))

    add_b1 = bool(np.any(b1_eff != 0))
    add_b2 = bool(np.any(b2_ != 0))
    nc = _build(rows, add_b2, add_b1)

    in_maps = []
    for c in range(N_CORES):
        sl = slice(c * rows, (c + 1) * rows)
        in_maps.append(
            {
                "cs": cs[sl],
                "nb": nb[sl],
                "e0": e0[sl],
                "e1": e1[sl],
                "e2": e2[sl],
                "w1": w1_dev,
                "b1x": b1x_dev,
                "b1p": b1p_dev,
                "w2": w2_eff,
                "b2t": b2_dev,
            }
        )

    trace = bool(int(os.environ.get("BASS_KERNEL_TRACE", "0")))
    res = run_bass_kernel_spmd(nc, in_maps, list(range(N_CORES)), trace=trace)
    if trace:
        kernel.last_results = res

    combined = np.concatenate([res.results[c]["comb"] for c in range(N_CORES)], 0)
    n_st = rows // ROWS_ST
    weights = np.concatenate(
        [
            res.results[c]["ew"]
            .reshape(P, n_st, NBLK, E)
            .transpose(1, 0, 2, 3)
            .reshape(rows, E)
            for c in range(N_CORES)
        ],
        0,
    )
    return combined, weights


# revision 22
# speedup vs baseline: 4.0230x; 1.2533x over previous
"""Trainium2 Bass kernel for MoEConnectionProcessor (moe_routing).

Per row b (S=256, H=64, E=3):
  normed = LayerNorm(current_state)                      # stats on DVE, apply on ACT
  h      = gelu([normed, neighbor] @ w1 + b1)            # PE (transpose + matmul), erf on ACT
  w      = softmax(h @ w2 + b2)                          # PE + tanh-based exp (one ACT table set)
  out    = sum_e w[e] * expert_out_e                     # DVE + GPSIMD

Data parallel over 8 cores: rows sharded on host, weights replicated.
gamma/beta are folded into w1/b1 on the host; gelu's 0.5 is folded into w2.
exp(x) is computed as (1+tanh(x/2))/(1-tanh(x/2)) so Copy/Erf/Tanh all live in
the single `sigmoid_and_others` ACT table set (no per-tile table reloads).
1/sqrt(var+eps) is a linear seed + 3 Newton iterations on DVE (no Sqrt table).
"""

import sys

if "/opt/trn_rl_repo" not in sys.path:
    sys.path.insert(0, "/opt/trn_rl_repo")

import os
from functools import lru_cache

import numpy as np

N_CORES = 8
S = 256          # feature dim
H = 64           # gate hidden
E = 3            # experts
LN_EPS = 1e-5
P = 128          # SBUF partitions = rows per block
NBLK = 4         # row-blocks per supertile
ROWS_ST = P * NBLK
NCHUNK = 2 * S // P  # K chunks for mm1 (= 4)

# linear minimax-ish seed for 1/sqrt(v) on v in [0.3, 2.2]; 3 NR iters after.
RSQRT_A = -0.6015
RSQRT_B = 1.7516


@lru_cache(maxsize=16)
def _build(
    rows_per_core: int, add_b2: bool, add_b1: bool, n_rep: int = 1, abl: int = 4,
    cmode: int = 2,
):
    # abl: ablation level for benchmarking. 1=DMA only, 2=+combine,
    # 3=+LayerNorm, 4=full (gate MLP + softmax). Only 4 is correct.
    import concourse.bass as bass
    import concourse.mybir as mybir
    from concourse import bacc, tile
    from concourse.masks import make_identity

    f32 = mybir.dt.float32
    AF = mybir.ActivationFunctionType
    ALU = mybir.AluOpType

    assert rows_per_core % ROWS_ST == 0
    n_st = rows_per_core // ROWS_ST

    nc = bacc.Bacc(
        "TRN2", target_bir_lowering=False, debug=False, num_devices=N_CORES
    )
    cs = nc.dram_tensor("cs", [rows_per_core, S], f32, kind="ExternalInput")
    nb = nc.dram_tensor("nb", [rows_per_core, S], f32, kind="ExternalInput")
    e0 = nc.dram_tensor("e0", [rows_per_core, S], f32, kind="ExternalInput")
    e1 = nc.dram_tensor("e1", [rows_per_core, S], f32, kind="ExternalInput")
    e2 = nc.dram_tensor("e2", [rows_per_core, S], f32, kind="ExternalInput")
    w1 = nc.dram_tensor("w1", [P, NCHUNK * H], f32, kind="ExternalInput")
    b1x = nc.dram_tensor("b1x", [H, 1], f32, kind="ExternalInput")
    b1p = nc.dram_tensor("b1p", [H, 1], f32, kind="ExternalInput")
    w2 = nc.dram_tensor("w2", [H, E], f32, kind="ExternalInput")
    b2t = nc.dram_tensor("b2t", [NBLK * E], f32, kind="ExternalInput")
    comb = nc.dram_tensor("comb", [rows_per_core, S], f32, kind="ExternalOutput")
    # p-major so the single end-of-kernel store is fully contiguous;
    # kernel() transposes back to row order on the host.
    ew = nc.dram_tensor(
        "ew", [P, (rows_per_core // ROWS_ST) * NBLK * E], f32,
        kind="ExternalOutput",
    )

    # row r = t*ROWS_ST + p*NBLK + n: each partition reads/writes one
    # contiguous NBLK*S*4-byte run per DMA. The host keeps the natural
    # row order because loads and stores use the same mapping.
    pat = "(t p n) f -> t p n f"
    csr = cs.rearrange(pat, n=NBLK, p=P)
    nbr = nb.rearrange(pat, n=NBLK, p=P)
    e0r = e0.rearrange(pat, n=NBLK, p=P)
    e1r = e1.rearrange(pat, n=NBLK, p=P)
    e2r = e2.rearrange(pat, n=NBLK, p=P)
    combr = comb.rearrange(pat, n=NBLK, p=P)


    with tile.TileContext(nc) as tc:
        from contextlib import ExitStack

        with ExitStack() as ctx:
            const = ctx.enter_context(tc.tile_pool(name="const", bufs=1))
            pin = ctx.enter_context(tc.tile_pool(name="pin", bufs=3))
            pnorm = ctx.enter_context(tc.tile_pool(name="pnorm", bufs=2))
            pxt = ctx.enter_context(tc.tile_pool(name="pxt", bufs=2))
            phg = ctx.enter_context(tc.tile_pool(name="phg", bufs=2))
            pout = ctx.enter_context(tc.tile_pool(name="pout", bufs=2))
            psmall = ctx.enter_context(tc.tile_pool(name="psmall", bufs=2))
            pcomb = ctx.enter_context(tc.tile_pool(name="pcomb", bufs=2))
            ppt = ctx.enter_context(tc.tile_pool(name="ppt", bufs=2, space="PSUM"))
            pph = ctx.enter_context(tc.tile_pool(name="pph", bufs=2, space="PSUM"))
            ppl = ctx.enter_context(tc.tile_pool(name="ppl", bufs=2, space="PSUM"))

            identity = const.tile([P, P], f32)
            make_identity(nc, identity)
            w1sb = const.tile([P, NCHUNK * H], f32)
            nc.sync.dma_start(out=w1sb, in_=w1[:, :])
            w2sb = const.tile([H, E], f32)
            nc.sync.dma_start(out=w2sb, in_=w2[:, :])
            b1xsb = const.tile([H, 1], f32)
            nc.sync.dma_start(out=b1xsb, in_=b1x[:, :])
            if add_b1:
                b1psb = const.tile([H, 1], f32)
                nc.sync.dma_start(out=b1psb, in_=b1p[:, :])
            if add_b2:
                b2sb = const.tile([P, NBLK, E], f32)
                b2_bcast = bass.AP(
                    tensor=b2t, offset=0, ap=[[0, P], [1, NBLK * E]]
                )
                nc.gpsimd.dma_start(out=b2sb, in_=b2_bcast)

            wnc = None
            if abl < 4:
                wnc = const.tile([P, NBLK, E], f32)
                nc.vector.memset(wnc, 0.33)
            wn_all = const.tile([P, n_st, NBLK, E], f32)
            if abl < 4:
                nc.vector.memset(wn_all, 0.0)

            from contextlib import nullcontext

            rep_ctx = tc.For_i(0, n_rep, 1) if n_rep > 1 else nullcontext()
            with rep_ctx:
                for t in range(n_st):
                    _supertile(
                        nc, tc, mybir, bass, t,
                        pin, pnorm, pxt, phg, pout, psmall, pcomb,
                        ppt, pph, ppl,
                        identity, w1sb, w2sb, b1xsb,
                        b1psb if add_b1 else None,
                        b2sb if add_b2 else None,
                        csr, nbr, e0r, e1r, e2r, combr, wn_all,
                        abl, wnc, cmode,
                    )
                nc.sync.dma_start(out=ew[:, :], in_=wn_all)

    nc.compile()
    return nc


def _supertile(
    nc, tc, mybir, bass, t,
    pin, pnorm, pxt, phg, pout, psmall, pcomb,
    ppt, pph, ppl,
    identity, w1sb, w2sb, b1xsb, b1psb, b2sb,
    csr, nbr, e0r, e1r, e2r, combr, wn_all,
    abl=4, wnc=None, cmode=2,
):
    f32 = mybir.dt.float32
    AF = mybir.ActivationFunctionType
    ALU = mybir.AluOpType
    add_b1 = b1psb is not None
    add_b2 = b2sb is not None
    if True:
            if True:
                xs = pin.tile([P, NBLK, S], f32, tag="xs")
                nc.sync.dma_start(out=xs, in_=csr[t])
                nbt = pin.tile([P, NBLK, S], f32, tag="nbt")
                nc.sync.dma_start(out=nbt, in_=nbr[t])
                et0 = pin.tile([P, NBLK, S], f32, tag="et0")
                nc.sync.dma_start(out=et0, in_=e0r[t])
                et1 = pin.tile([P, NBLK, S], f32, tag="et1")
                nc.sync.dma_start(out=et1, in_=e1r[t])
                et2 = pin.tile([P, NBLK, S], f32, tag="et2")
                nc.sync.dma_start(out=et2, in_=e2r[t])

                if abl <= 1:
                    # DMA only: store a load-dependent tile straight back
                    nc.sync.dma_start(out=combr[t], in_=et0)
                    return
                if abl == 2:
                    _combine(nc, mybir, t, pin, pout, pcomb, wnc,
                             et0, et1, et2, combr, cmode=cmode)
                    return

                # --- LayerNorm stats ---
                stats = psmall.tile([P, NBLK, 6], f32, tag="stats")
                # walrus requires exactly 6 output elems/partition per BNStats
                for n in range(NBLK):
                    nc.vector.bn_stats(out=stats[:, n], in_=xs[:, n])
                mv = psmall.tile([P, NBLK, 2], f32, tag="mv")
                for n in range(NBLK):
                    nc.vector.bn_aggr(out=mv[:, n], in_=stats[:, n])
                ve = psmall.tile([P, NBLK], f32, tag="ve")
                nc.vector.tensor_scalar_add(out=ve, in0=mv[:, :, 1], scalar1=LN_EPS)
                y = psmall.tile([P, NBLK], f32, tag="y")
                nc.vector.tensor_scalar(
                    out=y, in0=ve, scalar1=RSQRT_A, scalar2=RSQRT_B,
                    op0=ALU.mult, op1=ALU.add,
                )
                tnr = psmall.tile([P, NBLK], f32, tag="tnr")
                for _ in range(2):
                    nc.vector.tensor_mul(out=tnr, in0=y, in1=y)
                    nc.vector.tensor_mul(out=tnr, in0=tnr, in1=ve)
                    nc.vector.tensor_scalar(
                        out=tnr, in0=tnr, scalar1=-0.5, scalar2=1.5,
                        op0=ALU.mult, op1=ALU.add,
                    )
                    nc.vector.tensor_mul(out=y, in0=y, in1=tnr)
                nm = psmall.tile([P, NBLK], f32, tag="nm")
                nc.vector.tensor_scalar_mul(out=nm, in0=mv[:, :, 0], scalar1=-1.0)
                nc.vector.tensor_mul(out=nm, in0=nm, in1=y)

                # --- LN apply: normed = x*rstd + (-mu*rstd) on ACT ---
                normed = pnorm.tile([P, NBLK, S], f32, tag="normed")
                for n in range(NBLK):
                    nc.scalar.activation(
                        out=normed[:, n], in_=xs[:, n], func=AF.Identity,
                        bias=nm[:, n : n + 1], scale=y[:, n : n + 1],
                    )

                if abl == 3:
                    _combine(nc, mybir, t, pin, pout, pcomb, wnc,
                             et0, et1, et2, combr, extra_dep=normed,
                             cmode=cmode)
                    return

                # --- transpose [normed | neighbor] to feature-major ---
                xt = pxt.tile([P, NBLK, NCHUNK, P], f32, tag="xt")
                for n in range(NBLK):
                    pt = ppt.tile([P, NCHUNK, P], f32, tag="pt")
                    for k in range(NCHUNK):
                        if k < NCHUNK // 2:
                            src = normed[:, n, k * P : (k + 1) * P]
                        else:
                            kk = k - NCHUNK // 2
                            src = nbt[:, n, kk * P : (kk + 1) * P]
                        nc.tensor.transpose(pt[:, k], src, identity)
                    nc.scalar.activation(out=xt[:, n], in_=pt, func=AF.Copy)

                # --- mm1: h_pre^T [H, NBLK, P] ---
                ph = pph.tile([H, NBLK, P], f32, tag="ph")
                for k in range(NCHUNK):
                    nc.tensor.matmul(
                        ph, w1sb[:, k * H : (k + 1) * H], xt[:, :, k, :],
                        start=(k == 0), stop=(k == NCHUNK - 1),
                    )

                # --- gelu(x) = x*(1+erf(x/sqrt(2)))/2 ; the 0.5 is folded into w2 ---
                esb = phg.tile([H, NBLK, P], f32, tag="esb")
                nc.scalar.activation(
                    out=esb, in_=ph, func=AF.Erf, bias=b1xsb, scale=0.70710678
                )
                if add_b1:
                    xg = phg.tile([H, NBLK, P], f32, tag="xg")
                    nc.scalar.activation(out=xg, in_=ph, func=AF.Identity, bias=b1psb)
                    xsrc = xg
                else:
                    xsrc = ph
                tg = phg.tile([H, NBLK, P], f32, tag="tg")
                nc.vector.tensor_mul(out=tg, in0=xsrc, in1=esb)
                hg = phg.tile([H, NBLK, P], f32, tag="hg")
                nc.vector.tensor_add(out=hg, in0=tg, in1=xsrc)

                # --- mm2: logits [P, NBLK, E] ---
                pl = ppl.tile([P, NBLK, E], f32, tag="pl")
                for n in range(NBLK):
                    nc.tensor.matmul(
                        pl[:, n], hg[:, n, :], w2sb, start=True, stop=True
                    )

                # --- softmax via tanh-exp ---
                lcl = psmall.tile([P, NBLK, E], f32, tag="lcl")
                if add_b2:
                    nc.vector.tensor_add(out=lcl, in0=pl, in1=b2sb)
                    nc.vector.tensor_scalar(
                        out=lcl, in0=lcl, scalar1=16.0, scalar2=None, op0=ALU.min
                    )
                else:
                    nc.vector.tensor_scalar(
                        out=lcl, in0=pl, scalar1=16.0, scalar2=None, op0=ALU.min
                    )
                th = psmall.tile([P, NBLK, E], f32, tag="th")
                nc.scalar.activation(out=th, in_=lcl, func=AF.Tanh, scale=0.5)
                num = psmall.tile([P, NBLK, E], f32, tag="num")
                nc.vector.tensor_scalar_add(out=num, in0=th, scalar1=1.0)
                den = psmall.tile([P, NBLK, E], f32, tag="den")
                nc.vector.tensor_scalar(
                    out=den, in0=th, scalar1=-1.0, scalar2=1.0,
                    op0=ALU.mult, op1=ALU.add,
                )
                rden = psmall.tile([P, NBLK, E], f32, tag="rden")
                nc.vector.reciprocal(out=rden, in_=den)
                ex = psmall.tile([P, NBLK, E], f32, tag="ex")
                nc.vector.tensor_mul(out=ex, in0=num, in1=rden)
                ssum = psmall.tile([P, NBLK], f32, tag="ssum")
                nc.vector.tensor_reduce(
                    out=ssum, in_=ex, axis=mybir.AxisListType.X, op=ALU.add
                )
                rs = psmall.tile([P, NBLK], f32, tag="rs")
                nc.vector.reciprocal(out=rs, in_=ssum)
                wn = wn_all[:, t]
                rs_b = bass.AP(
                    tensor=rs.tensor, offset=rs.offset,
                    ap=[rs.ap[0], rs.ap[1], [0, E]],
                )
                nc.vector.tensor_tensor(
                    out=wn, in0=ex, in1=rs_b, op=ALU.mult
                )
                _combine(nc, mybir, t, pin, pout, pcomb, wn,
                         et0, et1, et2, combr, cmode=cmode)


def _combine(nc, mybir, t, pin, pout, pcomb, wn, et0, et1, et2,
             combr, extra_dep=None, cmode=2):
    f32 = mybir.dt.float32
    AF = mybir.ActivationFunctionType
    outc = pout.tile([P, NBLK, S], f32, tag="outc")
    m1 = pcomb.tile([P, NBLK, S], f32, tag="m1")
    m2 = pcomb.tile([P, NBLK, S], f32, tag="m2")
    for n in range(NBLK):
        # m1 = w1*E1, m2 = w2*E2 on a helper engine; m0 + adds on DVE
        if cmode == 0:
            nc.gpsimd.tensor_scalar_mul(
                out=m1[:, n], in0=et1[:, n], scalar1=wn[:, n, 1:2]
            )
            nc.gpsimd.tensor_scalar_mul(
                out=m2[:, n], in0=et2[:, n], scalar1=wn[:, n, 2:3]
            )
        elif cmode == 1:
            nc.vector.tensor_scalar_mul(
                out=m1[:, n], in0=et1[:, n], scalar1=wn[:, n, 1:2]
            )
            nc.vector.tensor_scalar_mul(
                out=m2[:, n], in0=et2[:, n], scalar1=wn[:, n, 2:3]
            )
        elif cmode == 2:
            nc.scalar.activation(
                out=m1[:, n], in_=et1[:, n], func=AF.Identity,
                scale=wn[:, n, 1:2],
            )
            nc.scalar.activation(
                out=m2[:, n], in_=et2[:, n], func=AF.Identity,
                scale=wn[:, n, 2:3],
            )
        elif cmode == 3:
            nc.scalar.activation(
                out=m1[:, n], in_=et1[:, n], func=AF.Identity,
                scale=wn[:, n, 1:2],
            )
            nc.gpsimd.tensor_scalar_mul(
                out=m2[:, n], in0=et2[:, n], scalar1=wn[:, n, 2:3]
            )
        src0 = et0[:, n] if extra_dep is None else extra_dep[:, n]
        nc.vector.tensor_scalar_mul(
            out=outc[:, n], in0=src0, scalar1=wn[:, n, 0:1]
        )
    nc.vector.tensor_add(out=outc, in0=outc, in1=m1)
    nc.vector.tensor_add(out=outc, in0=outc, in1=m2)
    nc.sync.dma_start(out=combr[t], in_=outc)


def kernel(
    current_state,
    neighbor_activity,
    expert_out_0,
    expert_out_1,
    expert_out_2,
    ln_gamma,
    ln_beta,
    w1,
    b1,
    w2,
    b2,
):
    from concourse.bass_utils import run_bass_kernel_spmd

    f32 = np.float32
    cs = np.ascontiguousarray(np.asarray(current_state, dtype=f32))
    nb = np.ascontiguousarray(np.asarray(neighbor_activity, dtype=f32))
    e0 = np.ascontiguousarray(np.asarray(expert_out_0, dtype=f32))
    e1 = np.ascontiguousarray(np.asarray(expert_out_1, dtype=f32))
    e2 = np.ascontiguousarray(np.asarray(expert_out_2, dtype=f32))
    gamma = np.asarray(ln_gamma, dtype=f32)
    beta = np.asarray(ln_beta, dtype=f32)
    w1_ = np.asarray(w1, dtype=f32)
    b1_ = np.asarray(b1, dtype=f32)
    w2_ = np.asarray(w2, dtype=f32)
    b2_ = np.asarray(b2, dtype=f32)

    B = cs.shape[0]
    assert B % (N_CORES * ROWS_ST) == 0, B
    rows = B // N_CORES

    # host-side folds (exact; gamma/beta fold into w1/b1, gelu's 0.5 into w2)
    s_dim = cs.shape[1]
    w1_eff = np.concatenate(
        [gamma[:, None] * w1_[:s_dim], w1_[s_dim:]], axis=0
    ).astype(f32)
    b1_eff = (b1_ + beta @ w1_[:s_dim]).astype(f32)
    w2_eff = (0.5 * w2_).astype(f32)
    # chunk-major layout for direct [P, NCHUNK*H] SBUF load
    w1_dev = np.ascontiguousarray(
        w1_eff.reshape(NCHUNK, P, H).transpose(1, 0, 2).reshape(P, NCHUNK * H)
    )
    b1x_dev = np.ascontiguousarray((b1_eff * np.float32(0.70710678))[:, None])
    b1p_dev = np.ascontiguousarray(b1_eff[:, None])
    b2_dev = np.ascontiguousarray(np.tile(b2_, NBLK))

    add_b1 = bool(np.any(b1_eff != 0))
    add_b2 = bool(np.any(b2_ != 0))
    nc = _build(rows, add_b2, add_b1)

    in_maps = []
    for c in range(N_CORES):
        sl = slice(c * rows, (c + 1) * rows)
        in_maps.append(
            {
                "cs": cs[sl],
                "nb": nb[sl],
                "e0": e0[sl],
                "e1": e1[sl],
                "e2": e2[sl],
                "w1": w1_dev,
                "b1x": b1x_dev,
                "b1p": b1p_dev,
                "w2": w2_eff,
                "b2t": b2_dev,
            }
        )

    trace = bool(int(os.environ.get("BASS_KERNEL_TRACE", "0")))
    res = run_bass_kernel_spmd(nc, in_maps, list(range(N_CORES)), trace=trace)
    if trace:
        kernel.last_results = res

    combined = np.concatenate([res.results[c]["comb"] for c in range(N_CORES)], 0)
    n_st = rows // ROWS_ST
    weights = np.concatenate(
        [
            res.results[c]["ew"]
            .reshape(P, n_st, NBLK, E)
            .transpose(1, 0, 2, 3)
            .reshape(rows, E)
            for c in range(N_CORES)
        ],
        0,
    )
    return combined, weights
